# revision 20
# baseline (speedup 1.0000x reference)
"""Bass SPMD kernel for nn_ConvRelu (lattice conv + relu) on 8 TRN2 NeuronCores.

v2 strategy (vs v1 baseline):
  - Gather table stored int8 (lv*32, clip +-127; 2^-5 scale folded into W):
    halves DMA descriptor bytes (64B rows) and SBUF traffic.
  - ONE merged indirect DMA per 8 subtiles (8192 descriptors) instead of one
    per (subtile, slot): SWDGE fixed overhead (~1us/instr) amortized 64x.
  - Center rows pre-transposed on host -> DMA'd straight in as the K=64
    matmul chunk; never touch the device transpose path.
  - Gathered rows int8 -> bf16 converted on DVE+Scalar, then transposed
    per-128-feature-chunk via a mix of XBAR dma_start_transpose (DMA engines)
    and PE identity-matmul transposes (load-balanced).
  - Matmuls batched N=512 (4 subtiles) per PSUM bank: 5 accumulating
    matmuls (4x K=128 neighbor chunks + 1x K=64 center chunk).
  - Output bf16 [64, n_pad]; host transposes/casts to f32.
"""
import os
from contextlib import ExitStack

import numpy as np
import ml_dtypes

import concourse.bass as bass
import concourse.tile as tile
from concourse.instruction_name_ordered_set import InstructionNameOrderedSet
from concourse import bacc, mybir
from concourse.bass_utils import run_bass_kernel_spmd

N_VERTICES = 500000
IN_CH = 64
NR_FILTERS = 64
FILTER_EXTENT = 9
N_CORES = 8
P = 128

SUB = int(os.environ.get("K_SUB", "32"))  # subtiles per hw-loop iteration
VERTS_PER_ITER = P * SUB
# The SWDGE indirect-DMA ucode consumes one 32B offset vector (8 int32) per
# partition per instruction; more than 8 offsets per partition reads a
# skewed/overlapping diagonal. So: exactly one subtile (8 slots x 128
# partitions = 1024 descriptors) per indirect DMA.
GATHER_SPLIT = int(os.environ.get("K_GSPLIT", str(SUB)))  # indirect DMAs/iter
GCOLS = int(os.environ.get("K_GCOLS", "1"))           # offsets/partition/gather
XB_PER_GROUP = int(os.environ.get("K_XB", "0"))       # XBAR subtiles per group
INT8_TABLE = os.environ.get("K_INT8", "0") == "1"     # int8 vs bf16 table

QSCALE = 32.0 if INT8_TABLE else 1.0  # int8 = round(lv*32); W folded 1/32


def _n_iters(n_core_verts):
    return -(-n_core_verts // VERTS_PER_ITER)


def build_program(n_rows, n_iters):
    n_pad = n_iters * VERTS_PER_ITER

    nc = bacc.Bacc("TRN2", target_bir_lowering=False, debug=False,
                   num_devices=N_CORES)

    lv8 = nc.dram_tensor("lv8", [n_rows, IN_CH],
                         mybir.dt.int8 if INT8_TABLE else mybir.dt.bfloat16,
                         kind="ExternalInput")
    lvct = nc.dram_tensor("lvct", [IN_CH, n_pad], mybir.dt.bfloat16,
                          kind="ExternalInput")
    nbr = nc.dram_tensor("nbr", [P, n_iters * SUB * 8], mybir.dt.int32,
                         kind="ExternalInput")
    wts = nc.dram_tensor("wts", [P, 5 * NR_FILTERS], mybir.dt.bfloat16,
                         kind="ExternalInput")
    bias = nc.dram_tensor("bias", [NR_FILTERS, 1], mybir.dt.float32,
                          kind="ExternalInput")
    ident = nc.dram_tensor("ident", [P, P], mybir.dt.bfloat16,
                           kind="ExternalInput")
    outT = nc.dram_tensor("outT", [NR_FILTERS, n_pad], mybir.dt.bfloat16,
                          kind="ExternalOutput")

    GB = SUB * 512            # gathered elements per partition per iter
    HB = GB // GATHER_SPLIT
    SUBS_PER_GATHER = SUB // GATHER_SPLIT

    with tile.TileContext(nc) as tc:
        with ExitStack() as ctx:
            const_p = ctx.enter_context(tc.tile_pool(name="const", bufs=1))
            idx_p = ctx.enter_context(tc.tile_pool(name="idx", bufs=2))
            raw_p = ctx.enter_context(tc.tile_pool(name="raw", bufs=2))
            rows_p = ctx.enter_context(tc.tile_pool(name="rows", bufs=2))
            ctr_p = ctx.enter_context(tc.tile_pool(name="ctr", bufs=2))
            rt_p = ctx.enter_context(tc.tile_pool(name="rt", bufs=3))
            ob_p = ctx.enter_context(tc.tile_pool(name="ob", bufs=2))
            pst_p = ctx.enter_context(
                tc.tile_pool(name="pst", bufs=6, space="PSUM"))
            pso_p = ctx.enter_context(
                tc.tile_pool(name="pso", bufs=2, space="PSUM"))

            # constants
            w_t = const_p.tile([P, 5 * NR_FILTERS], mybir.dt.bfloat16)
            nc.sync.dma_start(out=w_t[:], in_=wts.ap())
            b_t = const_p.tile([NR_FILTERS, 1], mybir.dt.float32)
            nc.sync.dma_start(out=b_t[:], in_=bias.ap())
            id_t = const_p.tile([P, P], mybir.dt.bfloat16)
            nc.sync.dma_start(out=id_t[:], in_=ident.ap())

            with tc.For_i(0, n_iters, 1) as it:
                # neighbor indices for this iteration: [128, 16*8]
                idx_t = idx_p.tile([P, SUB * 8], mybir.dt.int32)
                nc.sync.dma_start(out=idx_t[:],
                                  in_=nbr.ap()[:, bass.ts(it, SUB * 8)])

                # merged indirect gathers: row p of dest <- lv8[idx[p, j]]
                rows_t = rows_p.tile([P, GB], mybir.dt.bfloat16)
                if INT8_TABLE:
                    gdst = raw_p.tile([P, GB], mybir.dt.int8)
                else:
                    gdst = rows_t
                if GATHER_SPLIT == SUB and GCOLS < 8:
                    # sub-subtile gathers: GCOLS offsets per partition each
                    for s in range(SUB):
                        for h in range(8 // GCOLS):
                            c0 = s * 8 + h * GCOLS
                            nc.gpsimd.indirect_dma_start(
                                out=gdst[:, c0 * IN_CH:
                                         (c0 + GCOLS) * IN_CH],
                                out_offset=None,
                                in_=lv8.ap(),
                                in_offset=bass.IndirectOffsetOnAxis(
                                    ap=idx_t[:, c0:c0 + GCOLS], axis=0))
                else:
                    g_insts = []
                    for h in range(GATHER_SPLIT):
                        g_insts.append(nc.gpsimd.indirect_dma_start(
                            out=gdst[:, h * HB:(h + 1) * HB],
                            out_offset=None,
                            in_=lv8.ap(),
                            in_offset=bass.IndirectOffsetOnAxis(
                                ap=idx_t[:, h * SUBS_PER_GATHER * 8:
                                         (h + 1) * SUBS_PER_GATHER * 8],
                                axis=0)))
                    # Tail barrier: merged gathers (>128 descriptors) signal
                    # their completion sem on the FIRST 128-descriptor round.
                    # This last small gather's descriptors sit behind ALL
                    # prior rounds in every SDMA engine's FIFO ring, so its
                    # (correct, single-round) sem implies the whole
                    # iteration's gather traffic has drained. It rewrites the
                    # final slot column (idempotent), making every consumer
                    # of gdst's last region wait on it; the whole-tile
                    # convert below carries the barrier to all subtiles.
                    tail = nc.gpsimd.indirect_dma_start(
                        out=gdst[:, (SUB * 8 - 1) * IN_CH:SUB * 8 * IN_CH],
                        out_offset=None,
                        in_=lv8.ap(),
                        in_offset=bass.IndirectOffsetOnAxis(
                            ap=idx_t[:, SUB * 8 - 1:SUB * 8], axis=0))
                    # hard ordering edges so the tail's ring position is
                    # behind every merged gather's descriptors
                    deps = InstructionNameOrderedSet()
                    for gi in g_insts:
                        deps.add(gi.ins.name)
                    tail.ins.add_nosync_dependencies_from(deps)

                # center rows, pre-transposed on host: [64, 2048]
                ctr_t = ctr_p.tile([IN_CH, VERTS_PER_ITER], mybir.dt.bfloat16)
                nc.sync.dma_start(out=ctr_t[:],
                                  in_=lvct.ap()[:, bass.ts(it, VERTS_PER_ITER)])

                if INT8_TABLE:
                    # int8 -> bf16 upconvert on DVE. ONE whole-tile read so
                    # the dep set includes the tail-barrier gather above.
                    nc.vector.tensor_copy(out=rows_t[:], in_=gdst[:])

                ob_t = ob_p.tile([NR_FILTERS, VERTS_PER_ITER],
                                 mybir.dt.bfloat16)

                for g in range(SUB // 4):
                    rt_t = rt_p.tile([P, 4 * 512], mybir.dt.bfloat16)
                    rt3 = rt_t[:].rearrange("p (c n) -> p c n", c=4)
                    for j in range(4):
                        s = g * 4 + j
                        if j < XB_PER_GROUP:
                            # XBAR transpose: [128, 512] -> [128, 4, 128]
                            nc.sync.dma_start_transpose(
                                out=rt3[:, :, j * P:(j + 1) * P],
                                in_=rows_t[:, s * 512:(s + 1) * 512])
                        else:
                            for c in range(4):
                                ps_t = pst_p.tile([P, P], mybir.dt.bfloat16)
                                nc.tensor.transpose(
                                    out=ps_t[:],
                                    in_=rows_t[:, s * 512 + c * P:
                                               s * 512 + (c + 1) * P],
                                    identity=id_t[:])
                                eng = nc.vector if (c % 2 == 0) else nc.scalar
                                if eng is nc.vector:
                                    eng.tensor_copy(
                                        out=rt_t[:, c * 512 + j * P:
                                                 c * 512 + (j + 1) * P],
                                        in_=ps_t[:])
                                else:
                                    eng.copy(
                                        out=rt_t[:, c * 512 + j * P:
                                                 c * 512 + (j + 1) * P],
                                        in_=ps_t[:])

                    ps_o = pso_p.tile([NR_FILTERS, 512], mybir.dt.float32)
                    for m in range(4):
                        nc.tensor.matmul(
                            ps_o[:],
                            lhsT=w_t[:, m * NR_FILTERS:(m + 1) * NR_FILTERS],
                            rhs=rt_t[:, m * 512:(m + 1) * 512],
                            start=(m == 0), stop=False)
                    nc.tensor.matmul(
                        ps_o[:],
                        lhsT=w_t[0:IN_CH, 4 * NR_FILTERS:5 * NR_FILTERS],
                        rhs=ctr_t[:, g * 512:(g + 1) * 512],
                        start=False, stop=True)
                    nc.scalar.activation(
                        out=ob_t[:, g * 512:(g + 1) * 512], in_=ps_o[:],
                        func=mybir.ActivationFunctionType.Relu,
                        bias=b_t[:], scale=1.0)

                nc.sync.dma_start(
                    out=outT.ap()[:, bass.ts(it, VERTS_PER_ITER)],
                    in_=ob_t[:])

    nc.compile()
    return nc


def prep_shared(lv_np, w_np, b_np):
    """Host-side prep shared across cores."""
    if INT8_TABLE:
        lv8 = np.clip(np.round(lv_np * QSCALE), -127, 127).astype(np.int8)
    else:
        lv8 = lv_np.astype(ml_dtypes.bfloat16)
    w_s = (w_np / QSCALE).astype(np.float32)
    # wts[:, m*64:(m+1)*64] = neighbor chunk m (W rows 64+128m .. 64+128(m+1))
    # wts[0:64, 256:320]    = center chunk (W rows 0..64)
    wts = np.zeros((P, 5 * NR_FILTERS), dtype=np.float32)
    for m in range(4):
        wts[:, m * NR_FILTERS:(m + 1) * NR_FILTERS] = \
            w_s[IN_CH + m * P:IN_CH + (m + 1) * P, :]
    wts[0:IN_CH, 4 * NR_FILTERS:5 * NR_FILTERS] = w_s[0:IN_CH, :]
    return {
        "lv8": lv8,
        "wts": wts.astype(ml_dtypes.bfloat16),
        "bias": np.ascontiguousarray(
            b_np.astype(np.float32).reshape(-1, 1)),
        "ident": np.eye(P).astype(ml_dtypes.bfloat16),
    }


def prep_core_inputs(shared, lv_np, nbr_np, v0, v1, n_iters):
    n_pad = n_iters * VERTS_PER_ITER
    n_own = v1 - v0

    # centers, scaled by QSCALE (exact pow2), transposed: [64, n_pad]
    lvc = np.zeros((n_pad, IN_CH), dtype=np.float32)
    lvc[:n_own] = lv_np[v0:v1] * QSCALE
    lvct = np.ascontiguousarray(lvc.T).astype(ml_dtypes.bfloat16)

    nb = np.zeros((n_pad, 8), dtype=np.int32)
    nb[:n_own] = nbr_np[v0:v1].astype(np.int32)
    # nbr_pm[p, it*128 + s*8 + k] = nb[it*2048 + s*128 + p, k]
    nbr_pm = np.ascontiguousarray(
        nb.reshape(n_iters, SUB, P, 8).transpose(2, 0, 1, 3).reshape(P, -1))

    return dict(shared, lvct=lvct, nbr=nbr_pm)


def run(lv_np, nbr_np, w_np, b_np, trace=False):
    n_rows = lv_np.shape[0]
    per_core = n_rows // N_CORES
    n_iters = _n_iters(per_core)

    nc = build_program(n_rows, n_iters)

    shared = prep_shared(lv_np, w_np, b_np)
    in_maps = []
    for c in range(N_CORES):
        in_maps.append(prep_core_inputs(
            shared, lv_np, nbr_np, c * per_core, (c + 1) * per_core, n_iters))

    res = run_bass_kernel_spmd(nc, in_maps, core_ids=list(range(N_CORES)),
                               trace=trace)
    outs = []
    for c in range(N_CORES):
        oT = np.asarray(res.results[c]["outT"])  # [64, n_pad] bf16
        outs.append(oT[:, :per_core].T)
    full = np.concatenate(outs, axis=0).astype(np.float32)
    return full, res


def kernel(lv, neighbors, W, b):
    full, _ = run(np.asarray(lv), np.asarray(neighbors),
                  np.asarray(W), np.asarray(b), trace=False)
    return full
